# revision 5
# baseline (speedup 1.0000x reference)
"""KANConvTranspose2d forward on 8 Trainium2 NeuronCores.

Sharding: row-parallel over in_features (2304/8 = 288 per core).
Each core: b-splines for its 288 input features, scales+casts its weight
shard, accumulates partial [B, OUT_F] via PE matmuls (activations
stationary, weights streaming), then a ReduceScatter hands core c the
out-feature slice c*576..(c+1)*576 == output channel c, which it folds
locally to [B, 16, 16].
"""

import numpy as np

import concourse.bacc as bacc
import concourse.bass as bass
import concourse.mybir as mybir
import concourse.tile as tile
from concourse.bass_utils import run_bass_kernel_spmd

# module constants
CIN, COUT = 16, 8
HIN = WIN = 8
KK, ST, PD = 3, 2, 1
GRID_SIZE, SPLINE_ORDER = 5, 3
HOUT = WOUT = 16
OH_IN = OW_IN = 4
OH_OUT = OW_OUT = 8
IN_F = CIN * KK * KK * OH_IN * OW_IN        # 2304
OUT_F = COUT * KK * KK * OH_OUT * OW_OUT    # 4608
B = 64
NCORE = 8
IC = IN_F // NCORE                          # 288 in_features per core
OSH = OUT_F // NCORE                        # 576 out_features per core
NS = GRID_SIZE + SPLINE_ORDER               # 8 spline bases per feature
NG = GRID_SIZE + 2 * SPLINE_ORDER + 1       # 12 grid knots per feature

# per-core contraction chunking: 288 = 128 + 128 + 32
CHUNKS = [(0, 128), (128, 128), (256, 32)]
NBLK = 12                                   # out_features in 12 blocks of 384
BW = OUT_F // NBLK                          # 384

F32 = mybir.dt.float32
BF16 = mybir.dt.bfloat16

_CACHE = {}


def _build_bass():
    nc = bacc.Bacc("TRN2", target_bir_lowering=False, debug=False,
                   num_devices=NCORE)
    uT_d = nc.dram_tensor("uT", [IC, B], F32, kind="ExternalInput")
    g_d = nc.dram_tensor("grid", [IC, NG], F32, kind="ExternalInput")
    swT_d = nc.dram_tensor("swT", [NS, IC, OUT_F], F32, kind="ExternalInput")
    scT_d = nc.dram_tensor("scT", [IC, OUT_F], F32, kind="ExternalInput")
    bwT_d = nc.dram_tensor("bwT", [IC, OUT_F], F32, kind="ExternalInput")
    y_d = nc.dram_tensor("y", [B, HOUT * WOUT], F32, kind="ExternalOutput")
    # collective bounce buffers
    P_d = nc.dram_tensor("partial", [NCORE, B, OSH], F32)
    R_d = nc.dram_tensor("reduced", [B, OSH], F32)

    with tile.TileContext(nc) as tc:
        with (
            tc.tile_pool(name="const", bufs=1) as cpool,
            tc.tile_pool(name="btmp", bufs=1) as bpool,
            tc.tile_pool(name="scal", bufs=2) as spool,
            tc.tile_pool(name="win", bufs=3) as wpool,
            tc.tile_pool(name="wbf", bufs=3) as fpool,
            tc.tile_pool(name="epi", bufs=1) as epool,
            tc.tile_pool(name="psum", bufs=1, space="PSUM") as pspool,
        ):
            # ---------------- phase 1: b-splines per i-chunk ----------------
            bases_bf = []
            silu_bf = []
            for ci, (off, p) in enumerate(CHUNKS):
                u_t = cpool.tile([p, B], F32, tag=f"u{ci}")
                nc.sync.dma_start(out=u_t[:], in_=uT_d[off:off + p, :])
                g_t = cpool.tile([p, NG], F32, tag=f"g{ci}")
                nc.sync.dma_start(out=g_t[:], in_=g_d[off:off + p, :])

                # reciprocal knot spans per order k
                rd = {}
                for k in range(1, SPLINE_ORDER + 1):
                    L = NG - k
                    d_t = bpool.tile([p, L], F32, tag="dtmp")
                    nc.vector.tensor_tensor(
                        out=d_t[:], in0=g_t[:, k:NG], in1=g_t[:, 0:L],
                        op=mybir.AluOpType.subtract)
                    rd_t = cpool.tile([p, L], F32, tag=f"rd{k}_{ci}")
                    nc.vector.reciprocal(out=rd_t[:], in_=d_t[:])
                    rd[k] = rd_t

                # degree-0: ge[s] = (u >= g[s]); b0[s] = ge[s] - ge[s+1]
                ge = bpool.tile([p, NG, B], F32, tag="ge")
                nc.vector.tensor_tensor(
                    out=ge[:],
                    in0=u_t[:].unsqueeze(1).broadcast_to([p, NG, B]),
                    in1=g_t[:].unsqueeze(2).broadcast_to([p, NG, B]),
                    op=mybir.AluOpType.is_ge)
                b_prev = bpool.tile([p, NG - 1, B], F32, tag="b0")
                nc.vector.tensor_tensor(
                    out=b_prev[:], in0=ge[:, 0:NG - 1, :], in1=ge[:, 1:NG, :],
                    op=mybir.AluOpType.subtract)

                # de Boor recursion
                for k in range(1, SPLINE_ORDER + 1):
                    Lw = NG - k              # == len(b_prev)
                    w_t = bpool.tile([p, Lw, B], F32, tag=f"wt{k}")
                    nc.vector.tensor_tensor(
                        out=w_t[:],
                        in0=u_t[:].unsqueeze(1).broadcast_to([p, Lw, B]),
                        in1=g_t[:, 0:Lw].unsqueeze(2).broadcast_to([p, Lw, B]),
                        op=mybir.AluOpType.subtract)
                    nc.vector.tensor_tensor(
                        out=w_t[:], in0=w_t[:],
                        in1=rd[k][:].unsqueeze(2).broadcast_to([p, Lw, B]),
                        op=mybir.AluOpType.mult)
                    # P = W * b_prev (in place into w_t)
                    nc.vector.tensor_tensor(
                        out=w_t[:], in0=w_t[:], in1=b_prev[:],
                        op=mybir.AluOpType.mult)
                    b_new = bpool.tile([p, Lw - 1, B], F32, tag=f"b{k}")
                    # b_new[s] = P[s] + (b_prev[s+1] - P[s+1])
                    d2 = bpool.tile([p, Lw - 1, B], F32, tag=f"d{k}")
                    nc.vector.tensor_tensor(
                        out=d2[:], in0=b_prev[:, 1:Lw, :], in1=w_t[:, 1:Lw, :],
                        op=mybir.AluOpType.subtract)
                    nc.vector.tensor_tensor(
                        out=b_new[:], in0=w_t[:, 0:Lw - 1, :], in1=d2[:],
                        op=mybir.AluOpType.add)
                    b_prev = b_new

                bb = cpool.tile([p, NS, B], BF16, tag=f"bb{ci}")
                nc.vector.tensor_copy(out=bb[:], in_=b_prev[:])
                bases_bf.append(bb)

                si = cpool.tile([p, B], BF16, tag=f"si{ci}")
                nc.scalar.activation(si[:], u_t[:],
                                     mybir.ActivationFunctionType.Silu)
                silu_bf.append(si)

            # ---------------- phase 2: weight stream + matmul ----------------
            ps = [pspool.tile([128, BW], F32, tag=f"ps{b}", name=f"ps{b}")
                  for b in range(6)]
            pass_ix = 0
            nterm = len(CHUNKS) * (NS + 1)
            term_ix = 0
            for ci, (off, p) in enumerate(CHUNKS):
                sc_t = spool.tile([p, OUT_F], F32, tag="sc")
                nc.sync.dma_start(out=sc_t[:], in_=scT_d[off:off + p, :])
                for t in range(NS + 1):          # t==0: base path, else s=t-1
                    w_t = wpool.tile([p, OUT_F], F32, tag="w")
                    if t == 0:
                        nc.sync.dma_start(out=w_t[:],
                                          in_=bwT_d[off:off + p, :])
                    else:
                        nc.sync.dma_start(out=w_t[:],
                                          in_=swT_d[t - 1, off:off + p, :])
                    wb = fpool.tile([p, OUT_F], BF16, tag="wb")
                    eng = nc.vector if pass_ix % 3 != 2 else nc.gpsimd
                    pass_ix += 1
                    if t == 0:
                        eng.tensor_copy(out=wb[:], in_=w_t[:])
                        lhsT = silu_bf[ci][:]
                    else:
                        eng.tensor_tensor(out=wb[:], in0=w_t[:], in1=sc_t[:],
                                          op=mybir.AluOpType.mult)
                        lhsT = bases_bf[ci][:, t - 1, :]
                    start = term_ix == 0
                    stop = term_ix == nterm - 1
                    term_ix += 1
                    for blk in range(NBLK):
                        half, bank = divmod(blk, 6)
                        out_ap = ps[bank][half * B:(half + 1) * B, :]
                        nc.tensor.matmul(
                            out_ap, lhsT, wb[:, blk * BW:(blk + 1) * BW],
                            start=start, stop=stop,
                            tile_position=(0, 64 * half))

            # ---------------- phase 3: epilogue ----------------
            # y_sb rows 0-63: o[0:2304] for batch n; rows 64-127: o[2304:4608]
            y_sb = epool.tile([128, OUT_F // 2], F32, tag="ysb")
            for blk in range(NBLK):
                half, bank = divmod(blk, 6)
                nc.vector.tensor_copy(
                    out=y_sb[half * B:(half + 1) * B,
                             bank * BW:(bank + 1) * BW],
                    in_=ps[bank][half * B:(half + 1) * B, :])
            for h in range(2):
                nc.sync.dma_start(
                    out=P_d[h * 4:(h + 1) * 4].rearrange("s n j -> n s j"),
                    in_=y_sb[h * B:(h + 1) * B, :])
            nc.gpsimd.collective_compute(
                "ReduceScatter", mybir.AluOpType.add,
                replica_groups=[list(range(NCORE))],
                ins=[P_d[:]], outs=[R_d[:]])
            r_sb = epool.tile([B, KK * KK, OH_OUT * OW_OUT], F32, tag="rsb")
            nc.sync.dma_start(out=r_sb[:], in_=R_d[:])

            # fold: out_p[n, kh + 2*oh, kw + 2*ow] += r[n, (kh,kw), (oh,ow)]
            o_sb = epool.tile([B, HOUT + 2, WOUT + 2], F32, tag="osb")
            nc.vector.memset(o_sb[:], 0.0)
            for kk_ in range(KK * KK):
                kh, kw = divmod(kk_, KK)
                dst = o_sb[:, kh:kh + 2 * OH_OUT:2, kw:kw + 2 * OW_OUT:2]
                nc.vector.tensor_tensor(
                    out=dst, in0=dst,
                    in1=r_sb[:, kk_, :].rearrange(
                        "p (a b) -> p a b", a=OH_OUT),
                    op=mybir.AluOpType.add)
            nc.sync.dma_start(out=y_d[:],
                              in_=o_sb[:, 1:1 + HOUT, 1:1 + WOUT])

    nc.compile()
    return nc


def _unfold(x):
    xp = np.pad(x, ((0, 0), (0, 0), (PD, PD), (PD, PD)))
    pats = np.stack(
        [xp[:, :, i:i + (OH_IN - 1) * ST + 1:ST, j:j + (OW_IN - 1) * ST + 1:ST]
         for i in range(KK) for j in range(KK)], axis=2)
    return pats.reshape(B, CIN * KK * KK, OH_IN * OW_IN).reshape(B, IN_F)


def kernel(x, base_weight, spline_weight, spline_scaler, grid):
    if "nc" not in _CACHE:
        _CACHE["nc"] = _build_bass()
    nc = _CACHE["nc"]

    uT = np.ascontiguousarray(_unfold(np.asarray(x, np.float32)).T)  # [IN_F,B]
    swT = np.ascontiguousarray(
        np.asarray(spline_weight, np.float32).transpose(2, 1, 0))  # [NS,IN_F,OUT_F]
    scT = np.ascontiguousarray(np.asarray(spline_scaler, np.float32).T)
    bwT = np.ascontiguousarray(np.asarray(base_weight, np.float32).T)
    grid = np.ascontiguousarray(np.asarray(grid, np.float32))

    in_maps = []
    for c in range(NCORE):
        r0, r1 = c * IC, (c + 1) * IC
        in_maps.append({
            "uT": uT,
            "grid": grid[r0:r1],
            "swT": np.ascontiguousarray(swT[:, r0:r1, :]),
            "scT": np.ascontiguousarray(scT[r0:r1]),
            "bwT": np.ascontiguousarray(bwT[r0:r1]),
        })
    # every core needs only its own u rows for splines/silu
    for c in range(NCORE):
        in_maps[c]["uT"] = np.ascontiguousarray(uT[c * IC:(c + 1) * IC])

    res = run_bass_kernel_spmd(nc, in_maps, list(range(NCORE)))
    out = np.stack(
        [res.results[c]["y"].reshape(B, HOUT, WOUT) for c in range(NCORE)],
        axis=1)
    return np.ascontiguousarray(out.astype(np.float32))
